# revision 14
# baseline (speedup 1.0000x reference)
"""BoundaryAttentionModule Trainium2 kernel — centered moment expansion, fp8 DR.

Shapes (hardcoded): b=4, c=256, h=w=64 (HW=4096), mid=64, out_ch=256.
8 cores: core = (batch bi = core//2, key-half kh = core%2); each core
handles its 2048 keys against all 4096 queries j.

Math: E^T[k,j] = t_k*A_S[j] + B_S[j] within ReLU-region S of the scalar
boundary value t_k.  Expansion is CENTERED per region: with region
center t_S and half-width h_S, U[k,j] = exp(B'_S[j]) * exp(d A'_S[j])
where B' = B + t_S A, A' = h_S A, d = (t_k - t_S)/h_S in [-1,1].  The
host splits wide regions (64 region slots) so |d A'| is tiny and TWO
Taylor orders suffice: U ~ W0 + d*W1, W0 = exp(B'), W1 = W0*A'.
Host folds M = key_w2^T @ query_w into CA/CB: A'/B' come straight from
u via one fp8 DoubleRow matmul each (contraction c=256), no G2.

W [128=(n,S), 4096]: rows 0:64 = W0 = exp(B') (ACT exp from psum, with
sigma0 via accum), rows 64:128 = W1 = W0*A' (two scalar_tensor_tensor
[64,2048] steps on DVE, sigma1 via accum).  s = pwt^T @ sigma via 16
1-col matmuls; pws = PSCALE*pw/s in fp8; Mo via fp8 DR pair matmuls
over keys; P = Mo^T @ W in bf16; output fp8 (host divides PSCALE).
"""

import numpy as np

B, C, HW = 4, 256, 4096
KH = HW // 2          # 2048 keys per core
NKT = KH // 128       # 16 key tiles
RP = 64               # region slots
NORD = 2              # Taylor orders 0..1 (centered)
BASIS = NORD * RP     # 128
PSCALE = 128.0        # pws scale folded out on host via gamma

TRACE = False
TRACE_CORES = None
LAST_RESULTS = None

_BUILT = None


def _build():
    import concourse.bass as bass
    import concourse.tile as tile
    from concourse import bacc, mybir

    f32 = mybir.dt.float32
    bf16 = mybir.dt.bfloat16
    f8 = mybir.dt.float8e4
    AF = mybir.ActivationFunctionType
    AX = mybir.AxisListType
    ALU = mybir.AluOpType
    DR = mybir.MatmulPerfMode.DoubleRow

    nc = bacc.Bacc(
        "TRN2",
        target_bir_lowering=False,
        debug=False,
        enable_asserts=False,
        num_devices=8,
    )

    u8_in = nc.dram_tensor("u8_in", [128, 2, HW], f8, kind="ExternalInput").ap()
    cab8_in = nc.dram_tensor("cab8_in", [128, 2, 2 * RP], f8, kind="ExternalInput").ap()
    vw8_in = nc.dram_tensor("vw8_in", [128, 2, C], f8, kind="ExternalInput").ap()
    pwsb_in = nc.dram_tensor("pwsb_in", [128, NKT * BASIS], bf16, kind="ExternalInput").ap()
    pwt_in = nc.dram_tensor("pwt_in", [BASIS, KH], bf16, kind="ExternalInput").ap()
    p_out = nc.dram_tensor("p_out", [2, 128, HW], f8, kind="ExternalOutput").ap()

    with tile.TileContext(nc) as tc:
        with (
            tc.tile_pool(name="sb", bufs=1) as sb,
            tc.tile_pool(name="ab", bufs=2, space="PSUM") as abp,
            tc.tile_pool(name="vt", bufs=2, space="PSUM") as vtp,
            tc.tile_pool(name="pin", bufs=1, space="PSUM") as pinp,
        ):
            # ---- SBUF tiles ----
            u8 = sb.tile([128, 2, HW], f8, tag="u8", name="u8")
            cab8 = sb.tile([128, 2, 2 * RP], f8, tag="cab8", name="cab8")
            vw8 = sb.tile([128, 2, C], f8, tag="vw8", name="vw8")
            pwsb = sb.tile([128, NKT * BASIS], bf16, tag="pwsb", name="pwsb")
            pwsB = sb.tile([128, NKT * BASIS], bf16, tag="pwsB", name="pwsB")
            pwt = sb.tile([BASIS, KH], bf16, tag="pwt", name="pwt")
            Af = sb.tile([64, HW], bf16, tag="Af", name="Af")
            W = sb.tile([128, HW], bf16, tag="W", name="W")
            vtb = sb.tile([128, NKT * C], bf16, tag="vtb", name="vtb")
            sacc = sb.tile([64, 12], f32, tag="sacc", name="sacc")
            sigf = sb.tile([64, 2], f32, tag="sigf", name="sigf")
            sigb = sb.tile([128, 1], bf16, tag="sigb", name="sigb")
            rinv = sb.tile([128, NKT], f32, tag="rinv", name="rinv")
            mo0 = sb.tile([128, C], bf16, tag="mo0", name="mo0")
            po = sb.tile([128, 2 * HW], f8, tag="po", name="po")
            scr = sb.tile([128, 512], bf16, tag="scr", name="scr")
            nc.vector.memset(scr[:], 0.0)

            spin = pinp.tile([128, 512], f32, tag="spin", name="spin")
            s_ps = spin[:, 0:NKT]
            mo_ps = spin[:, 256 : 256 + C]

            # ---- input DMAs ----
            # Only the HW-DGE queues (sync/scalar) start promptly; gpsimd's
            # SW-DGE adds ~4us. c-half u slices are 4KB runs -> fast packets.
            nc.sync.dma_start(cab8[:], cab8_in[:, :, :])
            nc.sync.dma_start(vw8[:], vw8_in[:, :, :])
            nc.sync.dma_start(u8[:, 0:1, :], u8_in[:, 0:1, :])
            nc.scalar.dma_start(u8[:, 1:2, :], u8_in[:, 1:2, :])
            nc.scalar.dma_start(pwt[:], pwt_in[:, :])
            nc.scalar.dma_start(pwsb[:], pwsb_in[:, :])

            # ---- PE warm-up while inputs stream ----
            def warm(i, n=1):
                for k in range(n):
                    pwm = abp.tile([128, 512], f32, tag="pb", name=f"warm{i}_{k}")
                    nc.tensor.matmul(
                        pwm[:], scr[:, 0:128], scr[:, 0:512], start=True, stop=True
                    )

            warm("pre", 9)

            # ---- A|B matmul (one fp8 DR mm: out rows 0:64=A, 64:128=B),
            # exp from rows 64:128, Af copy from rows 0:64 ----
            def ab_chunk(ci, k):
                j0 = 512 * ci
                ptile = abp.tile([128, 512], f32, tag="pa", name=f"pab{ci}")
                nc.tensor.matmul(
                    ptile[:, 0:512], cab8[:, :, 0:128],
                    u8[:, :, j0 : j0 + 512],
                    start=True, stop=True, perf_mode=DR,
                )
                nc.scalar.activation(
                    W[0:64, j0 : j0 + 512], ptile[64:128, 0:512], AF.Exp,
                    accum_out=sacc[0:64, k : k + 1],
                )
                if k % 2 == 0:
                    nc.vector.tensor_copy(Af[0:64, j0 : j0 + 512], ptile[0:64, 0:512])
                else:
                    nc.scalar.copy(Af[0:64, j0 : j0 + 512], ptile[0:64, 0:512])

            def vt_pair(kt, dst_eng):
                pv = vtp.tile([128, 2 * C], f32, tag="pv", name=f"pv{kt}")
                for q in range(2):
                    nc.tensor.matmul(
                        pv[:, q * C : (q + 1) * C],
                        u8[:, :, (kt + q) * 128 : (kt + q + 1) * 128],
                        vw8[:, :, :],
                        start=True, stop=True, perf_mode=DR,
                    )
                dst = vtb[:, kt * C : (kt + 2) * C]
                if dst_eng is nc.scalar:
                    dst_eng.copy(dst, pv[:, 0 : 2 * C])
                else:
                    dst_eng.tensor_copy(dst, pv[:, 0 : 2 * C])

            # chunk order: j-halves interleaved so keys (cols 0:2048) and
            # chain inputs both complete early
            CHUNKS = (0, 4, 1, 5, 2, 6, 3, 7)
            for k, ci in enumerate(CHUNKS):
                ab_chunk(ci, k)
                if ci < 4:
                    vt_pair(4 * ci + 0, nc.vector if k % 2 else nc.scalar)
                    vt_pair(4 * ci + 2, nc.scalar if k % 2 else nc.vector)
                warm(f"ab{k}", 1)

            # ---- chain: W1 = W0 * A' (two [64,2048] steps, sigma1 accum) ----
            for half in range(2):
                j0 = half * KH
                nc.vector.scalar_tensor_tensor(
                    W[64:128, j0 : j0 + KH], W[0:64, j0 : j0 + KH], 1.0,
                    Af[0:64, j0 : j0 + KH],
                    op0=ALU.mult, op1=ALU.mult,
                    accum_out=sacc[0:64, 8 + half : 9 + half],
                )
            warm("ch", 6)

            # ---- sigma -> sigb [128,1]: rows 0:64 n0, 64:128 n1 ----
            nc.vector.reduce_sum(sigf[0:64, 0:1], sacc[0:64, 0:8], axis=AX.X)
            nc.vector.reduce_sum(sigf[0:64, 1:2], sacc[0:64, 8:10], axis=AX.X)
            nc.vector.tensor_copy(sigb[0:64, 0:1], sigf[0:64, 0:1])
            nc.vector.tensor_copy(sigb[64:128, 0:1], sigf[0:64, 1:2])

            # ---- s = pwt^T @ sigma ; rinv ----
            for kt in range(NKT):
                nc.tensor.matmul(
                    s_ps[:, kt : kt + 1],
                    pwt[:, kt * 128 : (kt + 1) * 128], sigb[:],
                    start=True, stop=True,
                )
            warm("s", 2)
            nc.vector.reciprocal(rinv[:], s_ps[:])

            # ---- pws = pwsb * rinv (bf16, DVE 4x), then moment (bf16) ----
            for kt in range(NKT):
                nc.vector.tensor_scalar(
                    pwsB[:, kt * BASIS : (kt + 1) * BASIS],
                    pwsb[:, kt * BASIS : (kt + 1) * BASIS],
                    rinv[:, kt : kt + 1], None, op0=ALU.mult,
                )
                nc.tensor.matmul(
                    mo_ps[:],
                    pwsB[:, kt * BASIS : (kt + 1) * BASIS],
                    vtb[:, kt * C : (kt + 1) * C],
                    start=(kt == 0), stop=(kt == NKT - 1),
                )
            nc.scalar.copy(mo0[:], mo_ps[:])

            # ---- P = Mo^T @ W -> fp8 out ----
            # psum ring alternates pa/pb tags (4-deep) so P matmuls never
            # stall on the copy drain; one contiguous 512KB DMA per c-block.
            for ct in range(2):
                for jg in range(8):
                    tg = "pa" if jg % 2 == 0 else "pb"
                    pp = abp.tile([128, 512], f32, tag=tg, name=f"pp{ct}_{jg}")
                    nc.tensor.matmul(
                        pp[:],
                        mo0[:, ct * 128 : (ct + 1) * 128],
                        W[:, jg * 512 : (jg + 1) * 512],
                        start=True, stop=True,
                    )
                    dst = po[:, ct * HW + jg * 512 : ct * HW + (jg + 1) * 512]
                    if jg % 2 == 0:
                        nc.scalar.copy(dst, pp[:])
                    else:
                        nc.vector.tensor_copy(dst, pp[:])
                if ct == 0:
                    nc.scalar.dma_start(p_out[0:1, :, :], po[:, 0:HW])
                else:
                    nc.sync.dma_start(p_out[1:2, 0:64, :], po[0:64, HW : 2 * HW])
                    nc.scalar.dma_start(p_out[1:2, 64:128, :], po[64:128, HW : 2 * HW])

    nc.compile()
    return nc


def _get_built():
    global _BUILT
    if _BUILT is None:
        _BUILT = _build()
    return _BUILT


def _regions(kw1f, beta, t):
    """Region edges: ReLU breakpoints inside t-range, merged to <= RP-1,
    then wide regions split so max |t - center| shrinks (all slots used)."""
    tmin, tmax = t.min(), t.max()
    bp = -beta / np.where(np.abs(kw1f) < 1e-30, 1e-30, kw1f)
    inr = np.sort(bp[(bp > tmin) & (bp < tmax)])
    while len(inr) > RP - 1:
        gaps = np.diff(np.concatenate([[tmin], inr, [tmax]]))
        i = int(np.argmin(gaps[:-1] + gaps[1:]))
        inr = np.delete(inr, i)
    edges = list(inr)
    while len(edges) < RP - 1:
        full = np.concatenate([[tmin - 1e-9], edges, [tmax + 1e-9]])
        bi, bm, bsplit = -1, -1.0, None
        for i in range(len(full) - 1):
            selm = t[(t > full[i]) & (t <= full[i + 1])]
            if len(selm) < 2:
                continue
            c = 0.5 * (selm.min() + selm.max())
            m = np.abs(selm - c).max()
            if m > bm:
                bm, bi, bsplit = m, i, float(np.median(selm))
        if bi < 0:
            break
        edges.append(bsplit)
        edges.sort()
    return np.array(edges)


def _host_prep(boundary_map, uncertainty_map, key_w1, bn_scale, bn_bias,
               bn_mean, bn_var, key_w2, query_w, value_w):
    import ml_dtypes

    bf = ml_dtypes.bfloat16
    f8 = ml_dtypes.float8_e4m3
    b, c, h, w = uncertainty_map.shape
    H0 = boundary_map.shape[2]
    idx = (np.arange(h) * H0) // h
    bm = boundary_map[:, 0][:, idx][:, :, idx].reshape(b, h * w).astype(np.float64)

    inv = bn_scale.astype(np.float64) / np.sqrt(bn_var.astype(np.float64) + 1e-5)
    beta = bn_bias.astype(np.float64) - bn_mean.astype(np.float64) * inv
    kw1f = key_w1[:, 0].astype(np.float64) * inv
    m_t = key_w2.T.astype(np.float64) @ query_w.astype(np.float64)   # [64, 256]
    vw_t = np.ascontiguousarray(value_w.T.astype(np.float64))        # [256, 256]
    vw8 = vw_t.reshape(2, 128, C).transpose(1, 0, 2)                 # [128,2,256]

    in_maps = []
    for core in range(8):
        bi, kh = core // 2, core % 2
        t_full = bm[bi]
        tk = t_full[kh * KH : (kh + 1) * KH]
        u = uncertainty_map[bi].reshape(c, h * w).astype(np.float64)
        u = np.roll(u, -kh * KH, axis=1)
        u8 = u.reshape(2, 128, HW).transpose(1, 0, 2)                # [128,2,HW]

        edges = _regions(kw1f, beta, tk)
        R = len(edges) + 1
        reg = np.searchsorted(edges, tk)                             # [KH]
        lo = np.concatenate([[tk.min() - 1e-9], edges])
        hi = np.concatenate([edges, [tk.max() + 1e-9]])
        relu_mid = 0.5 * (lo + hi)
        masks = (kw1f[None, :] * relu_mid[:, None] + beta[None, :]) > 0  # [R,64]
        tc = np.zeros(R)
        hh = np.ones(R)
        for r_ in range(R):
            selk = tk[reg == r_]
            if len(selk):
                tc[r_] = 0.5 * (selk.min() + selk.max())
                hh[r_] = max(np.abs(selk - tc[r_]).max(), 1e-6)
        ca = masks * kw1f[None, :]                                   # [R,64]
        cb = masks * beta[None, :]
        camT = ((hh[:, None] * ca) @ m_t).T                          # [256, R]
        cbmT = ((cb + tc[:, None] * ca) @ m_t).T                     # [256, R]
        cabf = np.zeros((256, 2 * RP))
        cabf[:, 0:R] = camT
        cabf[:, RP : RP + R] = cbmT
        cab8 = cabf.reshape(2, 128, 2 * RP).transpose(1, 0, 2)       # [128,2,128]

        dlt = (tk - tc[reg]) / hh[reg]                               # [KH]
        pw = np.zeros((KH, BASIS))
        krange = np.arange(KH)
        pw[krange, reg] = 1.0
        pw[krange, RP + reg] = dlt
        # pwsb [128 kw, kt*BASIS+bc] = PSCALE * pw[kt*128+kw, bc]
        # (PSCALE folded back out on host via gamma)
        pwsb = (PSCALE * pw).reshape(NKT, 128, BASIS).transpose(1, 0, 2).reshape(
            128, NKT * BASIS)
        in_maps.append({
            "u8_in": np.ascontiguousarray(u8).astype(f8),
            "cab8_in": np.ascontiguousarray(cab8).astype(f8),
            "vw8_in": np.ascontiguousarray(vw8).astype(f8),
            "pwsb_in": np.ascontiguousarray(pwsb).astype(bf),
            "pwt_in": np.ascontiguousarray(pw.T).astype(bf),
        })
    return in_maps


def kernel(boundary_map, uncertainty_map, key_w1, bn_scale, bn_bias,
           bn_mean, bn_var, key_w2, query_w, value_w, gamma):
    global LAST_RESULTS
    from concourse.bass_utils import run_bass_kernel_spmd

    nc = _get_built()
    in_maps = _host_prep(
        np.asarray(boundary_map), np.asarray(uncertainty_map), np.asarray(key_w1),
        np.asarray(bn_scale), np.asarray(bn_bias), np.asarray(bn_mean),
        np.asarray(bn_var), np.asarray(key_w2), np.asarray(query_w),
        np.asarray(value_w),
    )
    kwargs = {}
    if TRACE:
        kwargs["trace"] = True
        if TRACE_CORES is not None:
            kwargs["trace_cores"] = TRACE_CORES
    res = run_bass_kernel_spmd(nc, in_maps, core_ids=list(range(8)), **kwargs)
    LAST_RESULTS = res

    b, c, h, w = uncertainty_map.shape
    g = np.float64(np.asarray(gamma).reshape(-1)[0]) / PSCALE
    out = np.empty((b, c, h * w), np.float32)
    um = np.asarray(uncertainty_map)
    for bi in range(b):
        P = (res.results[2 * bi]["p_out"].astype(np.float32).reshape(C, HW)
             + np.roll(res.results[2 * bi + 1]["p_out"].astype(np.float32).reshape(C, HW),
                       KH, axis=1))
        out[bi] = g * P + um[bi].reshape(c, h * w)
    return out.reshape(b, c, h, w)


# revision 15
# speedup vs baseline: 1.0668x; 1.0668x over previous
"""BoundaryAttentionModule Trainium2 kernel — centered moment expansion, fp8 DR.

Shapes (hardcoded): b=4, c=256, h=w=64 (HW=4096), mid=64, out_ch=256.
8 cores: core = (batch bi = core//2, key-half kh = core%2); each core
handles its 2048 keys against all 4096 queries j.

Math: E^T[k,j] = t_k*A_S[j] + B_S[j] within ReLU-region S of the scalar
boundary value t_k.  Expansion is CENTERED per region: with region
center t_S and half-width h_S, U[k,j] = exp(B'_S[j]) * exp(d A'_S[j])
where B' = B + t_S A, A' = h_S A, d = (t_k - t_S)/h_S in [-1,1].  The
host splits wide regions (64 region slots) so |d A'| is tiny and TWO
Taylor orders suffice: U ~ W0 + d*W1, W0 = exp(B'), W1 = W0*A'.
Host folds M = key_w2^T @ query_w into CA/CB: A'/B' come straight from
u via one fp8 DoubleRow matmul each (contraction c=256), no G2.

W [128=(n,S), 4096]: rows 0:64 = W0 = exp(B') (ACT exp from psum, with
sigma0 via accum), rows 64:128 = W1 = W0*A' (two scalar_tensor_tensor
[64,2048] steps on DVE, sigma1 via accum).  s = pwt^T @ sigma via 16
1-col matmuls; pws = PSCALE*pw/s in fp8; Mo via fp8 DR pair matmuls
over keys; P = Mo^T @ W in bf16; output fp8 (host divides PSCALE).
"""

import numpy as np

B, C, HW = 4, 256, 4096
KH = HW // 2          # 2048 keys per core
NKT = KH // 128       # 16 key tiles
RP = 64               # region slots
NORD = 2              # Taylor orders 0..1 (centered)
BASIS = NORD * RP     # 128
PSCALE = 128.0        # pws scale folded out on host via gamma

TRACE = False
TRACE_CORES = None
LAST_RESULTS = None

_BUILT = None


def _build():
    import concourse.bass as bass
    import concourse.tile as tile
    from concourse import bacc, mybir

    f32 = mybir.dt.float32
    bf16 = mybir.dt.bfloat16
    f8 = mybir.dt.float8e4
    AF = mybir.ActivationFunctionType
    AX = mybir.AxisListType
    ALU = mybir.AluOpType
    DR = mybir.MatmulPerfMode.DoubleRow

    nc = bacc.Bacc(
        "TRN2",
        target_bir_lowering=False,
        debug=False,
        enable_asserts=False,
        num_devices=8,
    )

    u8_in = nc.dram_tensor("u8_in", [128, 2, HW], f8, kind="ExternalInput").ap()
    cab8_in = nc.dram_tensor("cab8_in", [128, 2, 2 * RP], f8, kind="ExternalInput").ap()
    vw8_in = nc.dram_tensor("vw8_in", [128, 2, C], f8, kind="ExternalInput").ap()
    pwsb_in = nc.dram_tensor("pwsb_in", [128, NKT * BASIS], bf16, kind="ExternalInput").ap()
    pwt_in = nc.dram_tensor("pwt_in", [BASIS, KH], bf16, kind="ExternalInput").ap()
    p_out = nc.dram_tensor("p_out", [2, 128, HW], f8, kind="ExternalOutput").ap()

    with tile.TileContext(nc) as tc:
        with (
            tc.tile_pool(name="sb", bufs=1) as sb,
            tc.tile_pool(name="ab", bufs=2, space="PSUM") as abp,
            tc.tile_pool(name="vt", bufs=2, space="PSUM") as vtp,
            tc.tile_pool(name="pin", bufs=1, space="PSUM") as pinp,
        ):
            # ---- SBUF tiles ----
            u8 = sb.tile([128, 2, HW], f8, tag="u8", name="u8")
            cab8 = sb.tile([128, 2, 2 * RP], f8, tag="cab8", name="cab8")
            vw8 = sb.tile([128, 2, C], f8, tag="vw8", name="vw8")
            pwsb = sb.tile([128, NKT * BASIS], bf16, tag="pwsb", name="pwsb")
            pwsB = sb.tile([128, NKT * BASIS], bf16, tag="pwsB", name="pwsB")
            pwt = sb.tile([BASIS, KH], bf16, tag="pwt", name="pwt")
            Af = sb.tile([64, HW], bf16, tag="Af", name="Af")
            W = sb.tile([128, HW], bf16, tag="W", name="W")
            vtb = sb.tile([128, NKT * C], bf16, tag="vtb", name="vtb")
            sacc = sb.tile([64, 12], f32, tag="sacc", name="sacc")
            sigf = sb.tile([64, 2], f32, tag="sigf", name="sigf")
            sigb = sb.tile([128, 2], bf16, tag="sigb", name="sigb")
            rinv = sb.tile([128, NKT], f32, tag="rinv", name="rinv")
            mo0 = sb.tile([128, C], bf16, tag="mo0", name="mo0")
            po = sb.tile([128, 2 * HW], f8, tag="po", name="po")
            scr = sb.tile([128, 512], bf16, tag="scr", name="scr")
            nc.vector.memset(scr[:], 0.0)

            spin = pinp.tile([128, 512], f32, tag="spin", name="spin")
            s_ps = spin[:, 0:NKT]
            mo_ps = spin[:, 256 : 256 + C]

            # ---- input DMAs ----
            # Only the HW-DGE queues (sync/scalar) start promptly; gpsimd's
            # SW-DGE adds ~4us. c-half u slices are 4KB runs -> fast packets.
            nc.gpsimd.dma_start(cab8[:], cab8_in[:, :, :])
            nc.gpsimd.dma_start(vw8[:], vw8_in[:, :, :])
            nc.sync.dma_start(u8[:, 0:1, :], u8_in[:, 0:1, :])
            nc.scalar.dma_start(u8[:, 1:2, :], u8_in[:, 1:2, :])
            nc.scalar.dma_start(pwt[:], pwt_in[:, :])
            nc.scalar.dma_start(pwsb[:], pwsb_in[:, :])

            # ---- PE warm-up while inputs stream ----
            def warm(i, n=1):
                for k in range(n):
                    pwm = abp.tile([128, 512], f32, tag="pb", name=f"warm{i}_{k}")
                    nc.tensor.matmul(
                        pwm[:], scr[:, 0:128], scr[:, 0:512], start=True, stop=True
                    )

            warm("pre", 9)

            # ---- A|B matmul (one fp8 DR mm: out rows 0:64=A, 64:128=B),
            # exp from rows 64:128, Af copy from rows 0:64 ----
            def ab_chunk(ci, k):
                j0 = 512 * ci
                ptile = abp.tile([128, 512], f32, tag="pa", name=f"pab{ci}")
                nc.tensor.matmul(
                    ptile[:, 0:512], cab8[:, :, 0:128],
                    u8[:, :, j0 : j0 + 512],
                    start=True, stop=True, perf_mode=DR,
                )
                nc.scalar.activation(
                    W[0:64, j0 : j0 + 512], ptile[64:128, 0:512], AF.Exp,
                    accum_out=sacc[0:64, k : k + 1],
                )
                if k % 2 == 0:
                    nc.vector.tensor_copy(Af[0:64, j0 : j0 + 512], ptile[0:64, 0:512])
                else:
                    nc.scalar.copy(Af[0:64, j0 : j0 + 512], ptile[0:64, 0:512])

            def vt_pair(kt, dst_eng):
                pv = vtp.tile([128, 2 * C], f32, tag="pv", name=f"pv{kt}")
                for q in range(2):
                    nc.tensor.matmul(
                        pv[:, q * C : (q + 1) * C],
                        u8[:, :, (kt + q) * 128 : (kt + q + 1) * 128],
                        vw8[:, :, :],
                        start=True, stop=True, perf_mode=DR,
                    )
                dst = vtb[:, kt * C : (kt + 2) * C]
                if dst_eng is nc.scalar:
                    dst_eng.copy(dst, pv[:, 0 : 2 * C])
                else:
                    dst_eng.tensor_copy(dst, pv[:, 0 : 2 * C])

            # chunk order: j-halves interleaved so keys (cols 0:2048) and
            # chain inputs both complete early
            CHUNKS = (0, 4, 1, 5, 2, 6, 3, 7)
            for k, ci in enumerate(CHUNKS):
                ab_chunk(ci, k)
                if ci < 4:
                    vt_pair(4 * ci + 0, nc.vector if k % 2 else nc.scalar)
                    vt_pair(4 * ci + 2, nc.scalar if k % 2 else nc.vector)
                warm(f"ab{k}", 1)

            # ---- chain halves + per-half sigma -> accumulated s matmuls ----
            # exp accum col k is j-half (k%2) [CHUNKS interleaves halves], so
            # each half's s contribution runs as soon as its chain step ends.
            for half in range(2):
                j0 = half * KH
                nc.vector.scalar_tensor_tensor(
                    W[64:128, j0 : j0 + KH], W[0:64, j0 : j0 + KH], 1.0,
                    Af[0:64, j0 : j0 + KH],
                    op0=ALU.mult, op1=ALU.mult,
                    accum_out=sacc[0:64, 8 + half : 9 + half],
                )
                nc.vector.reduce_sum(
                    sigf[0:64, half : half + 1],
                    sacc[0:64, half : 8 : 2], axis=AX.X,
                )
                nc.vector.tensor_copy(
                    sigb[0:64, half : half + 1], sigf[0:64, half : half + 1]
                )
                nc.vector.tensor_copy(
                    sigb[64:128, half : half + 1], sacc[0:64, 8 + half : 9 + half]
                )
                for kt in range(NKT):
                    nc.tensor.matmul(
                        s_ps[:, kt : kt + 1],
                        pwt[:, kt * 128 : (kt + 1) * 128],
                        sigb[:, half : half + 1],
                        start=(half == 0), stop=(half == 1),
                    )
                warm(f"sh{half}", 2)
            nc.vector.reciprocal(rinv[:], s_ps[:])

            # ---- pws = pwsb * rinv (bf16, DVE 4x), then moment (bf16) ----
            for kt in range(NKT):
                nc.vector.tensor_scalar(
                    pwsB[:, kt * BASIS : (kt + 1) * BASIS],
                    pwsb[:, kt * BASIS : (kt + 1) * BASIS],
                    rinv[:, kt : kt + 1], None, op0=ALU.mult,
                )
                nc.tensor.matmul(
                    mo_ps[:],
                    pwsB[:, kt * BASIS : (kt + 1) * BASIS],
                    vtb[:, kt * C : (kt + 1) * C],
                    start=(kt == 0), stop=(kt == NKT - 1),
                )
            nc.scalar.copy(mo0[:], mo_ps[:])
            warm("mo", 1)

            # ---- P = Mo^T @ W -> fp8 out ----
            # psum ring alternates pa/pb tags (4-deep) so P matmuls never
            # stall on the copy drain; one contiguous 512KB DMA per c-block.
            for ct in range(2):
                for jg in range(8):
                    tg = "pa" if jg % 2 == 0 else "pb"
                    pp = abp.tile([128, 512], f32, tag=tg, name=f"pp{ct}_{jg}")
                    nc.tensor.matmul(
                        pp[:],
                        mo0[:, ct * 128 : (ct + 1) * 128],
                        W[:, jg * 512 : (jg + 1) * 512],
                        start=True, stop=True,
                    )
                    dst = po[:, ct * HW + jg * 512 : ct * HW + (jg + 1) * 512]
                    if jg % 2 == 0:
                        nc.scalar.copy(dst, pp[:])
                    else:
                        nc.vector.tensor_copy(dst, pp[:])
                if ct == 0:
                    nc.gpsimd.dma_start(p_out[0:1, :, :], po[:, 0:HW])
                else:
                    nc.sync.dma_start(p_out[1:2, 0:64, :], po[0:64, HW : 2 * HW])
                    nc.scalar.dma_start(p_out[1:2, 64:128, :], po[64:128, HW : 2 * HW])

    nc.compile()
    return nc


def _get_built():
    global _BUILT
    if _BUILT is None:
        _BUILT = _build()
    return _BUILT


def _regions(kw1f, beta, t):
    """Region edges: ReLU breakpoints inside t-range, merged to <= RP-1,
    then wide regions split so max |t - center| shrinks (all slots used)."""
    tmin, tmax = t.min(), t.max()
    bp = -beta / np.where(np.abs(kw1f) < 1e-30, 1e-30, kw1f)
    inr = np.sort(bp[(bp > tmin) & (bp < tmax)])
    while len(inr) > RP - 1:
        gaps = np.diff(np.concatenate([[tmin], inr, [tmax]]))
        i = int(np.argmin(gaps[:-1] + gaps[1:]))
        inr = np.delete(inr, i)
    edges = list(inr)
    while len(edges) < RP - 1:
        full = np.concatenate([[tmin - 1e-9], edges, [tmax + 1e-9]])
        bi, bm, bsplit = -1, -1.0, None
        for i in range(len(full) - 1):
            selm = t[(t > full[i]) & (t <= full[i + 1])]
            if len(selm) < 2:
                continue
            c = 0.5 * (selm.min() + selm.max())
            m = np.abs(selm - c).max()
            if m > bm:
                bm, bi, bsplit = m, i, float(np.median(selm))
        if bi < 0:
            break
        edges.append(bsplit)
        edges.sort()
    return np.array(edges)


def _host_prep(boundary_map, uncertainty_map, key_w1, bn_scale, bn_bias,
               bn_mean, bn_var, key_w2, query_w, value_w):
    import ml_dtypes

    bf = ml_dtypes.bfloat16
    f8 = ml_dtypes.float8_e4m3
    b, c, h, w = uncertainty_map.shape
    H0 = boundary_map.shape[2]
    idx = (np.arange(h) * H0) // h
    bm = boundary_map[:, 0][:, idx][:, :, idx].reshape(b, h * w).astype(np.float64)

    inv = bn_scale.astype(np.float64) / np.sqrt(bn_var.astype(np.float64) + 1e-5)
    beta = bn_bias.astype(np.float64) - bn_mean.astype(np.float64) * inv
    kw1f = key_w1[:, 0].astype(np.float64) * inv
    m_t = key_w2.T.astype(np.float64) @ query_w.astype(np.float64)   # [64, 256]
    vw_t = np.ascontiguousarray(value_w.T.astype(np.float64))        # [256, 256]
    vw8 = vw_t.reshape(2, 128, C).transpose(1, 0, 2)                 # [128,2,256]

    in_maps = []
    for core in range(8):
        bi, kh = core // 2, core % 2
        t_full = bm[bi]
        tk = t_full[kh * KH : (kh + 1) * KH]
        u = uncertainty_map[bi].reshape(c, h * w).astype(np.float64)
        u = np.roll(u, -kh * KH, axis=1)
        u8 = u.reshape(2, 128, HW).transpose(1, 0, 2)                # [128,2,HW]

        edges = _regions(kw1f, beta, tk)
        R = len(edges) + 1
        reg = np.searchsorted(edges, tk)                             # [KH]
        lo = np.concatenate([[tk.min() - 1e-9], edges])
        hi = np.concatenate([edges, [tk.max() + 1e-9]])
        relu_mid = 0.5 * (lo + hi)
        masks = (kw1f[None, :] * relu_mid[:, None] + beta[None, :]) > 0  # [R,64]
        tc = np.zeros(R)
        hh = np.ones(R)
        for r_ in range(R):
            selk = tk[reg == r_]
            if len(selk):
                tc[r_] = 0.5 * (selk.min() + selk.max())
                hh[r_] = max(np.abs(selk - tc[r_]).max(), 1e-6)
        ca = masks * kw1f[None, :]                                   # [R,64]
        cb = masks * beta[None, :]
        camT = ((hh[:, None] * ca) @ m_t).T                          # [256, R]
        cbmT = ((cb + tc[:, None] * ca) @ m_t).T                     # [256, R]
        cabf = np.zeros((256, 2 * RP))
        cabf[:, 0:R] = camT
        cabf[:, RP : RP + R] = cbmT
        cab8 = cabf.reshape(2, 128, 2 * RP).transpose(1, 0, 2)       # [128,2,128]

        dlt = (tk - tc[reg]) / hh[reg]                               # [KH]
        pw = np.zeros((KH, BASIS))
        krange = np.arange(KH)
        pw[krange, reg] = 1.0
        pw[krange, RP + reg] = dlt
        # pwsb [128 kw, kt*BASIS+bc] = PSCALE * pw[kt*128+kw, bc]
        # (PSCALE folded back out on host via gamma)
        pwsb = (PSCALE * pw).reshape(NKT, 128, BASIS).transpose(1, 0, 2).reshape(
            128, NKT * BASIS)
        in_maps.append({
            "u8_in": np.ascontiguousarray(u8).astype(f8),
            "cab8_in": np.ascontiguousarray(cab8).astype(f8),
            "vw8_in": np.ascontiguousarray(vw8).astype(f8),
            "pwsb_in": np.ascontiguousarray(pwsb).astype(bf),
            "pwt_in": np.ascontiguousarray(pw.T).astype(bf),
        })
    return in_maps


def kernel(boundary_map, uncertainty_map, key_w1, bn_scale, bn_bias,
           bn_mean, bn_var, key_w2, query_w, value_w, gamma):
    global LAST_RESULTS
    from concourse.bass_utils import run_bass_kernel_spmd

    nc = _get_built()
    in_maps = _host_prep(
        np.asarray(boundary_map), np.asarray(uncertainty_map), np.asarray(key_w1),
        np.asarray(bn_scale), np.asarray(bn_bias), np.asarray(bn_mean),
        np.asarray(bn_var), np.asarray(key_w2), np.asarray(query_w),
        np.asarray(value_w),
    )
    kwargs = {}
    if TRACE:
        kwargs["trace"] = True
        if TRACE_CORES is not None:
            kwargs["trace_cores"] = TRACE_CORES
    res = run_bass_kernel_spmd(nc, in_maps, core_ids=list(range(8)), **kwargs)
    LAST_RESULTS = res

    b, c, h, w = uncertainty_map.shape
    g = np.float64(np.asarray(gamma).reshape(-1)[0]) / PSCALE
    out = np.empty((b, c, h * w), np.float32)
    um = np.asarray(uncertainty_map)
    for bi in range(b):
        P = (res.results[2 * bi]["p_out"].astype(np.float32).reshape(C, HW)
             + np.roll(res.results[2 * bi + 1]["p_out"].astype(np.float32).reshape(C, HW),
                       KH, axis=1))
        out[bi] = g * P + um[bi].reshape(c, h * w)
    return out.reshape(b, c, h, w)
